# revision 8
# baseline (speedup 1.0000x reference)
"""DigitCaps (capsule routing) Trainium2 kernel, 8-core SPMD.

Shard N=1152 primary capsules across 8 cores (144 each); B=256, C=10
replicated. Einsum u_hat = einsum('bni,cnio->cbno') runs on the PE via
block-diagonal W tiles; routing runs on DVE/ACT with n on the free axis;
cross-shard sums (s_j, softmax Z) via AllReduce.
"""
import numpy as np

C, B, N, I, O = 10, 256, 1152, 8, 16
NCORES = 8
NL = N // NCORES          # 144 n per core
NBLK = NL // 16           # 9 blocks of 16 n
BH = 2                    # two batch halves of 128
P = 128

_cache = {}


def _build():
    import concourse.bacc as bacc
    import concourse.mybir as mybir
    from concourse.tile import TileContext
    from concourse.masks import make_identity

    f32 = mybir.dt.float32
    Alu = mybir.AluOpType
    Act = mybir.ActivationFunctionType

    nc = bacc.Bacc("TRN2", target_bir_lowering=False, debug=False,
                   num_devices=NCORES)

    u_sh = nc.dram_tensor("u_sh", [B, NL, I], f32, kind="ExternalInput")
    w_sh = nc.dram_tensor("w_sh", [C, NL, I, O], f32, kind="ExternalInput")
    uhat_out = nc.dram_tensor("uhat_out", [C, B, NL, O], f32, kind="ExternalOutput")
    a_out = nc.dram_tensor("a_out", [C, B, NL], f32, kind="ExternalOutput")
    v_out = nc.dram_tensor("v_out", [C, B, O], f32, kind="ExternalOutput")

    with TileContext(nc) as tc:
        with (
            tc.tile_pool(name="const", bufs=1) as consts,
            tc.tile_pool(name="wnat", bufs=1) as wpool,
            tc.tile_pool(name="ut", bufs=1) as utpool,
            tc.tile_pool(name="uhat", bufs=11) as uhpool,
            tc.tile_pool(name="prodp", bufs=2) as prodp,
            tc.tile_pool(name="unatp", bufs=1) as unatp,
            tc.tile_pool(name="wbdp", bufs=3) as wbdp,
            tc.tile_pool(name="astp", bufs=1) as astp,
            tc.tile_pool(name="small", bufs=2) as small,
            tc.tile_pool(name="psum_tr", bufs=2, space="PSUM") as psum_tr,
            tc.tile_pool(name="psum", bufs=4, space="PSUM") as psum,
            tc.tile_pool(name="psum_s0", bufs=2, space="PSUM") as psum_s0,
            tc.tile_pool(name="dram", bufs=2, space="DRAM") as dram,
        ):
            # ---- constants ----
            ident = consts.tile([P, P], f32, tag="ident")
            make_identity(nc, ident[:])
            mask = consts.tile([P, 16, O], f32, tag="mask")
            nc.gpsimd.memset(mask[:], 0.0)
            # keep (=1.0) where 0 <= p - 8*nsub <= 7
            nc.gpsimd.affine_select(out=mask[:], in_=mask[:], compare_op=Alu.is_gt,
                                    fill=1.0, base=-7, pattern=[[-8, 16], [0, O]],
                                    channel_multiplier=1)
            nc.gpsimd.affine_select(out=mask[:], in_=mask[:], compare_op=Alu.is_ge,
                                    fill=0.0, base=0, pattern=[[-8, 16], [0, O]],
                                    channel_multiplier=1)

            # ---- W natural tiles: [(nsub,i)=128, j=9, o=16] per class ----
            w_nat = []
            for c in range(C):
                wt = wpool.tile([P, NBLK, O], f32, tag=f"wnat{c}")
                nc.sync.dma_start(
                    out=wt[:],
                    in_=w_sh[c].rearrange("(j n2) i o -> (n2 i) j o", j=NBLK))
                w_nat.append(wt)

            # ---- u_t tiles: per (bh, j): [(nsub,i)=128, b=128] ----
            u_t = {}
            for bh in range(BH):
                unat = unatp.tile([P, NL * I], f32, tag="unat")
                nc.sync.dma_start(
                    out=unat[:],
                    in_=u_sh[bh * P:(bh + 1) * P].rearrange("b n i -> b (n i)"))
                for j in range(NBLK):
                    ps = psum_tr.tile([P, P], f32, tag="ps_tr")
                    nc.tensor.transpose(ps[:], unat[:, j * P:(j + 1) * P], ident[:])
                    ut = utpool.tile([P, P], f32, tag=f"ut{bh}_{j}")
                    nc.vector.tensor_copy(ut[:], ps[:])
                    u_t[(bh, j)] = ut

            # ---- state tiles ----
            a0_st, a1_st, uhat_sb = {}, {}, {}

            def einsum_bh(bh):
                """einsum + s0 partial for one batch half. Returns s0_sb [p,(c,o)]."""
                s0_sb = small.tile([P, C, O], f32, tag="s0sb")
                for c in range(C):
                    uh = uhpool.tile([P, NL, O], f32, tag="uhat")
                    uhat_sb[(bh, c)] = uh
                    ps0 = psum_s0.tile([P, O], f32, tag="ps_s0")
                    for j in range(NBLK):
                        wbd = wbdp.tile([P, 16, O], f32, tag="wbd")
                        nc.vector.tensor_tensor(
                            out=wbd[:],
                            in0=w_nat[c][:, j, :].unsqueeze(1).broadcast_to([P, 16, O]),
                            in1=mask[:], op=Alu.mult)
                        ps = psum.tile([P, 16 * O], f32, tag="ps_mm")
                        nc.tensor.matmul(ps[:], u_t[(bh, j)][:],
                                         wbd[:].rearrange("p n o -> p (n o)"),
                                         start=True, stop=True)
                        nc.tensor.matmul(ps0[:], u_t[(bh, j)][:], w_nat[c][:, j, :],
                                         start=(j == 0), stop=(j == NBLK - 1))
                        eng = nc.vector if (j % 2 == 0) else nc.scalar
                        if eng is nc.vector:
                            nc.vector.tensor_copy(
                                uh[:, j * 16:(j + 1) * 16, :],
                                ps[:].rearrange("p (n o) -> p n o", o=O))
                        else:
                            nc.scalar.copy(
                                uh[:, j * 16:(j + 1) * 16, :],
                                ps[:].rearrange("p (n o) -> p n o", o=O))
                    nc.vector.tensor_copy(s0_sb[:, c, :], ps0[:])
                    nc.sync.dma_start(out=uhat_out[c, bh * P:(bh + 1) * P], in_=uh[:])
                return s0_sb

            def allreduce(sb_ap, width, tag):
                """AllReduce an SBUF [P, width] f32 AP across all 8 cores."""
                bi = dram.tile([P, width], f32, tag="arin")
                bo = dram.tile([P, width], f32, tag="arout")
                nc.sync.dma_start(out=bi[:], in_=sb_ap)
                nc.gpsimd.collective_compute(
                    "AllReduce", Alu.add,
                    replica_groups=[list(range(NCORES))],
                    ins=[bi.opt()], outs=[bo.opt()])
                g = small.tile([P, width], f32, tag="arg")
                nc.sync.dma_start(out=g[:], in_=bo[:])
                return g

            def squash(s3, tag):
                """v = s*sqrt(sq)/(1+sq) per (c) group; s3: AP [P,C,O] -> v [P,C,O]."""
                sq_t = small.tile([P, C, O], f32, tag="sqs")
                nc.scalar.square(sq_t[:], s3)
                sq = small.tile([P, C], f32, tag="sq")
                nc.vector.tensor_reduce(out=sq[:], in_=sq_t[:],
                                        axis=mybir.AxisListType.X, op=Alu.add)
                r = small.tile([P, C], f32, tag="r")
                nc.scalar.sqrt(r[:], sq[:])
                d = small.tile([P, C], f32, tag="d")
                nc.vector.tensor_scalar_add(d[:], sq[:], 1.0)
                rd = small.tile([P, C], f32, tag="rd")
                nc.vector.reciprocal(rd[:], d[:])
                sc = small.tile([P, C], f32, tag="sc")
                nc.vector.tensor_tensor(out=sc[:], in0=r[:], in1=rd[:], op=Alu.mult)
                v = small.tile([P, C, O], f32, tag="v")
                nc.vector.tensor_tensor(
                    out=v[:], in0=s3,
                    in1=sc[:].unsqueeze(-1).broadcast_to([P, C, O]), op=Alu.mult)
                return v

            def a_pass(bh, v, tag_base):
                """a[c,b,n] = sum_o u_hat*v -> a_st [P, C, NL]."""
                a_st = astp.tile([P, C, NL], f32, tag=tag_base)
                for c in range(C):
                    prod = prodp.tile([P, NL, O], f32, tag="prod")
                    nc.vector.tensor_tensor(
                        out=prod[:], in0=uhat_sb[(bh, c)][:],
                        in1=v[:, c, :].unsqueeze(1).broadcast_to([P, NL, O]),
                        op=Alu.mult)
                    nc.vector.tensor_reduce(out=a_st[:, c, :], in_=prod[:],
                                            axis=mybir.AxisListType.X, op=Alu.add)
                return a_st

            def stilde_pass(bh, beta_st, tag):
                """e = exp(beta); s~_partial = sum_n e*u_hat ; Z_partial = sum_n e.
                Returns sZ_sb [P, C*O + C] (s~ then Z)."""
                sZ = small.tile([P, C * O + C], f32, tag="sZt")
                for c in range(C):
                    e = small.tile([P, NL], f32, tag="e")
                    nc.scalar.activation(out=e[:], in_=beta_st[:, c, :], func=Act.Exp,
                                         accum_out=sZ[:, C * O + c:C * O + c + 1])
                    prod = prodp.tile([P, NL, O], f32, tag="prod")
                    nc.vector.tensor_tensor(
                        out=prod[:], in0=uhat_sb[(bh, c)][:],
                        in1=e[:].unsqueeze(-1).broadcast_to([P, NL, O]), op=Alu.mult)
                    nc.vector.tensor_reduce(
                        out=sZ[:, c * O:(c + 1) * O],
                        in_=prod[:].transpose([0, 2, 1]),
                        axis=mybir.AxisListType.X, op=Alu.add)
                return sZ

            def norm_and_squash(sZ_g, tag):
                """s = s~/Z ; return squash(s)."""
                zr = small.tile([P, C], f32, tag="zr")
                nc.vector.reciprocal(zr[:], sZ_g[:, C * O:])
                s = small.tile([P, C, O], f32, tag="sn")
                nc.vector.tensor_tensor(
                    out=s[:], in0=sZ_g[:, :C * O].rearrange("p (c o) -> p c o", o=O),
                    in1=zr[:].unsqueeze(-1).broadcast_to([P, C, O]), op=Alu.mult)
                return squash(s[:], tag)

            for bh in range(BH):
                s0_sb = einsum_bh(bh)
                s0_g = allreduce(s0_sb[:].rearrange("p c o -> p (c o)"), C * O, f"s0_{bh}")
                # s0 mean = sum/N ; then squash
                s0m = small.tile([P, C * O], f32, tag="s0m")
                nc.vector.tensor_scalar_mul(s0m[:], s0_g[:], 1.0 / N)
                v0 = squash(s0m[:].rearrange("p (c o) -> p c o", o=O), f"v0_{bh}")

                a0 = a_pass(bh, v0, "ast_a0")
                a0_st[bh] = a0

                sZ1 = stilde_pass(bh, a0, f"s1_{bh}")
                sZ1_g = allreduce(sZ1[:], C * O + C, f"s1_{bh}")
                v1 = norm_and_squash(sZ1_g, f"v1_{bh}")

                a1 = a_pass(bh, v1, "ast_a1")
                a1_st[bh] = a1
                nc.sync.dma_start(
                    out=a_out[:, bh * P:(bh + 1) * P, :].transpose([1, 0, 2]),
                    in_=a1[:])

                nc.vector.tensor_tensor(out=a0[:], in0=a0[:], in1=a1[:], op=Alu.add)
                sZ2 = stilde_pass(bh, a0, f"s2_{bh}")
                sZ2_g = allreduce(sZ2[:], C * O + C, f"s2_{bh}")
                v2 = norm_and_squash(sZ2_g, f"v2_{bh}")
                nc.sync.dma_start(
                    out=v_out[:, bh * P:(bh + 1) * P, :].transpose([1, 0, 2]),
                    in_=v2[:])

    nc.compile()
    return nc


def kernel(u, W):
    u = np.ascontiguousarray(np.asarray(u, dtype=np.float32))
    W = np.ascontiguousarray(np.asarray(W, dtype=np.float32))
    if "nc" not in _cache:
        _cache["nc"] = _build()
    nc = _cache["nc"]

    from concourse.bass_utils import run_bass_kernel_spmd

    in_maps = []
    for k in range(NCORES):
        sl = slice(k * NL, (k + 1) * NL)
        in_maps.append({
            "u_sh": np.ascontiguousarray(u[:, sl, :]),
            "w_sh": np.ascontiguousarray(W[:, sl, :, :]),
        })
    res = run_bass_kernel_spmd(nc, in_maps, list(range(NCORES)))

    uhat = np.concatenate([res.results[k]["uhat_out"] for k in range(NCORES)], axis=2)
    a = np.concatenate([res.results[k]["a_out"] for k in range(NCORES)], axis=2)
    v = res.results[0]["v_out"]

    return (v[:, :, None, None, :],
            a[:, :, :, None, None],
            uhat[:, :, :, None, :])
